# revision 19
# baseline (speedup 1.0000x reference)
"""Distributed sparse-MoE (top-1 routing, shared expert FFN) for 8 trn2 NeuronCores.

Math: reference computes
    logits = hidden @ Wg + bg ; probs = softmax(logits)
    best   = argmax(probs)    ; order = stable argsort(best)
    out[t] = (hidden[order[t]] @ We + be) * probs[t, best[t]]

Since every expert shares the same FFN weight `We`, the dispatch permutation
commutes with the matmul:  (hidden[order]) @ We = (hidden @ We)[order].
So each core runs the dense FFN matmul on a contiguous 2048-token shard in
ORIGINAL token order (no all-to-all needed), with the router gate computed in
the same K-loop as 8 extra fp32 output columns.  The host applies the
data-dependent permutation + top-1 probability scale while gathering the 8
shards back into the full output.

Device work per core: [2048, 2048] @ [2048, 2056] (+bias) -> [2048, 2056].
The big FFN matmul runs in float32r (tf32, full PE rate); the gate columns
run in plain fp32 because argmax tie-breaking vs the reference needs logit
error < 6.7e-5 (measured min top-2 gap 1.3e-4).
"""

import os

import numpy as np

import concourse.bacc as bacc
import concourse.bass as bass
import concourse.mybir as mybir
import concourse.tile as tile
from concourse.bass_utils import run_bass_kernel_spmd

# Problem shape (hardcoded per contract).
B, S, H, E = 4, 4096, 2048, 8
T = B * S            # 16384 tokens
NCORES = 8
TPC = T // NCORES    # 2048 tokens per core
P = 128              # partitions
KT = H // P          # 16 contraction blocks
NW = 512             # matmul moving free-dim (one PSUM bank of fp32)
NMAIN = H // NW      # 4 main n-groups
SLAB = 128           # tokens per x DMA slab (= one m-tile)

# Main-matmul dtype: "f32r" (tf32, full PE rate), "f32" (4x slower, exact),
# "bf16" (full rate, ~4e-3 rel err).  Gate always plain fp32.
DT_MAIN = os.environ.get("MOE_DT", "f32r")
# "device": gate ridden along in the main K-loop.  "host": numpy fp32 gate.
GATE = os.environ.get("MOE_GATE", "device")


def _round_tf32(a: np.ndarray) -> np.ndarray:
    """Round fp32 to tf32 (10-bit mantissa), round-to-nearest-even."""
    u = np.ascontiguousarray(a, dtype=np.float32).view(np.uint32)
    r = (u + np.uint32(0xFFF) + ((u >> np.uint32(13)) & np.uint32(1))) & np.uint32(
        0xFFFFE000
    )
    return r.view(np.float32)


def _build(dt_main: str, gate_device: bool) -> bass.Bass:
    # Bacc (not raw Bass): its compile() runs generate_event_semaphores,
    # which splits multi-waits to satisfy TRN2's 1-wait-per-instruction
    # hardware constraint.
    nc = bacc.Bacc(None, target_bir_lowering=False)
    f32 = mybir.dt.float32
    f32r = mybir.dt.float32r
    bf16 = mybir.dt.bfloat16
    mm_dt = {"f32r": f32r, "f32": f32, "bf16": bf16}[dt_main]
    nout = H + (E if gate_device else 0)

    # xr: tokens pre-rounded on host to the matmul dtype's precision
    # (tf32 for f32r; plain f32 otherwise).  bf16 ships a bf16 copy.
    xr = nc.dram_tensor("xr", [H, TPC], mm_dt, kind="ExternalInput")
    wm = nc.dram_tensor("wm", [H, H], mm_dt, kind="ExternalInput")
    bc = nc.dram_tensor("bc", [1, nout], f32, kind="ExternalInput")
    if gate_device:
        # Exact-f32 tokens for the router gate (argmax tie safety).
        xt = nc.dram_tensor("xt", [H, TPC], f32, kind="ExternalInput")
        wg = nc.dram_tensor("wg", [H, E], f32, kind="ExternalInput")
    else:
        sc = nc.dram_tensor("sc", [TPC, 1], f32, kind="ExternalInput")
    yo = nc.dram_tensor("yo", [TPC, nout], f32, kind="ExternalOutput")

    xr_r = xr[:].rearrange("(ko ki) t -> ki ko t", ki=P)   # [128, KT, TPC]
    wm_r = wm[:].rearrange("(ko ki) n -> ki ko n", ki=P)   # [128, KT, H]

    with tile.TileContext(nc) as tc:
        with (
            tc.tile_pool(name="wpool", bufs=1) as wpool,
            tc.tile_pool(name="cpool", bufs=1) as cpool,
            tc.tile_pool(name="xpool", bufs=2) as xpool,
            tc.tile_pool(name="rpool", bufs=2) as rpool,
            tc.tile_pool(name="gpool", bufs=2) as gpool,
            tc.tile_pool(name="opool", bufs=3) as opool,
            tc.tile_pool(name="spool", bufs=2) as spool,
            tc.tile_pool(name="pspool", bufs=5, space="PSUM") as pspool,
            tc.tile_pool(name="psgpool", bufs=2, space="PSUM") as psgpool,
            tc.tile_pool(name="psdpool", bufs=1, space="PSUM") as psdpool,
        ):
            # Bias row replicated to all 128 partitions via 0-stride DMA.
            b_sb = cpool.tile([P, nout], f32)
            bias_bcast = bass.AP(tensor=bc, offset=0, ap=[[0, P], [1, nout]])
            nc.sync.dma_start(out=b_sb, in_=bias_bcast)

            # Preload W, n-chunked so PE can start right after chunk 0 lands.
            w_sb = wpool.tile([P, KT, H], mm_dt)
            for n in range(NMAIN):
                nc.sync.dma_start(
                    out=w_sb[:, :, n * NW : (n + 1) * NW],
                    in_=wm_r[:, :, n * NW : (n + 1) * NW],
                )
            if gate_device:
                wg_sb = wpool.tile([P, KT, E], f32)
                nc.sync.dma_start(
                    out=wg_sb, in_=wg[:].rearrange("(ko ki) e -> ki ko e", ki=P)
                )

            n_slabs = TPC // SLAB
            for m in range(n_slabs):
                xm = rpool.tile([P, KT, SLAB], mm_dt, tag="xm")
                nc.sync.dma_start(out=xm, in_=xr_r[:, :, m * SLAB : (m + 1) * SLAB])
                if gate_device:
                    xs = xpool.tile([P, KT, SLAB], f32, tag="xs")
                    nc.sync.dma_start(
                        out=xs,
                        in_=xt[:].rearrange("(ko ki) t -> ki ko t", ki=P)[
                            :, :, m * SLAB : (m + 1) * SLAB
                        ],
                    )
                else:
                    s_m = spool.tile([P, 1], f32, tag="s")
                    nc.sync.dma_start(out=s_m, in_=sc[m * P : (m + 1) * P, :])
                for n in range(NMAIN):
                    ps = pspool.tile([P, NW], f32, tag="ps")
                    for k in range(KT):
                        nc.tensor.matmul(
                            ps,
                            xm[:, k, :],
                            w_sb[:, k, n * NW : (n + 1) * NW],
                            start=(k == 0),
                            stop=(k == KT - 1),
                        )
                    o_sb = opool.tile([P, NW], f32, tag="o")
                    nc.vector.tensor_add(
                        out=o_sb, in0=ps, in1=b_sb[:, n * NW : (n + 1) * NW]
                    )
                    if not gate_device:
                        nc.vector.tensor_scalar_mul(
                            out=o_sb, in0=o_sb, scalar1=s_m
                        )
                    nc.sync.dma_start(
                        out=yo[m * P : (m + 1) * P, n * NW : (n + 1) * NW],
                        in_=o_sb,
                    )
                if gate_device:
                    psg = psgpool.tile([P, E], f32, tag="psg")
                    for k in range(KT):
                        nc.tensor.matmul(
                            psg,
                            xs[:, k, :],
                            wg_sb[:, k, :],
                            start=(k == 0),
                            stop=(k == KT - 1),
                        )
                    og = opool.tile([P, E], f32, tag="og")
                    nc.vector.tensor_add(out=og, in0=psg, in1=b_sb[:, H:])
                    nc.sync.dma_start(out=yo[m * P : (m + 1) * P, H:], in_=og)
    nc.compile()
    return nc


_NC_CACHE: dict = {}


def _get_nc(dt_main: str, gate_device: bool) -> bass.Bass:
    key = (dt_main, gate_device)
    if key not in _NC_CACHE:
        _NC_CACHE[key] = _build(dt_main, gate_device)
    return _NC_CACHE[key]


def _softmax_top1(logits: np.ndarray):
    """best index, top-1 softmax prob (fp32, matches jax argmax semantics)."""
    mx = logits.max(axis=1, keepdims=True)
    ex = np.exp(logits - mx, dtype=np.float32)
    denom = ex.sum(axis=1)
    best = logits.argmax(axis=1)
    best_p = ex[np.arange(logits.shape[0]), best] / denom
    return best, best_p


def _prep_mm(a: np.ndarray, dt_main: str) -> np.ndarray:
    """Prepare an operand for the main matmul's dtype (host-side rounding)."""
    if dt_main == "f32r":
        return _round_tf32(a)
    if dt_main == "bf16":
        import ml_dtypes

        return np.ascontiguousarray(a).astype(ml_dtypes.bfloat16)
    return np.ascontiguousarray(a)


def kernel(x, Wg, bg, We, be):
    x = np.asarray(x, dtype=np.float32)
    Wg = np.asarray(Wg, dtype=np.float32)
    bg = np.asarray(bg, dtype=np.float32)
    We = np.asarray(We, dtype=np.float32)
    be = np.asarray(be, dtype=np.float32)

    hidden = np.ascontiguousarray(x.reshape(T, H))
    gate_device = GATE == "device"
    nc = _get_nc(DT_MAIN, gate_device)
    wm_np = _prep_mm(We, DT_MAIN)

    if gate_device:
        bc_full = np.concatenate([be, bg])[None, :].astype(np.float32)
        in_maps = []
        for c in range(NCORES):
            xt_c = np.ascontiguousarray(hidden[c * TPC : (c + 1) * TPC].T)
            in_maps.append(
                {
                    "xr": _prep_mm(xt_c, DT_MAIN),
                    "xt": xt_c,
                    "wm": wm_np,
                    "wg": Wg,
                    "bc": bc_full,
                }
            )
        res = run_bass_kernel_spmd(nc, in_maps, core_ids=list(range(NCORES)))
        yo = np.concatenate([r["yo"] for r in res.results], axis=0)  # [T, H+E]
        y = yo[:, :H]
        logits = yo[:, H:]
        best, best_p = _softmax_top1(logits)
        order = np.argsort(best, kind="stable")
        out = y[order] * best_p[:, None]
    else:
        # Host gate: shards are the tokens PERMUTED by destination slot; the
        # device applies the top-1 scale, so shard outputs are final rows.
        logits = hidden @ Wg + bg
        best, best_p = _softmax_top1(logits)
        order = np.argsort(best, kind="stable")
        xp = hidden[order]
        bc_full = be[None, :].astype(np.float32)
        in_maps = []
        for c in range(NCORES):
            xt_c = np.ascontiguousarray(xp[c * TPC : (c + 1) * TPC].T)
            sc_c = np.ascontiguousarray(best_p[c * TPC : (c + 1) * TPC, None])
            in_maps.append(
                {
                    "xr": _prep_mm(xt_c, DT_MAIN),
                    "wm": wm_np,
                    "bc": bc_full,
                    "sc": sc_c,
                }
            )
        res = run_bass_kernel_spmd(nc, in_maps, core_ids=list(range(NCORES)))
        out = np.concatenate([r["yo"] for r in res.results], axis=0)

    return out.reshape(B, S, H).astype(np.float32)
